# Initial kernel scaffold
#
"""Trainium2 Bass kernel for the low-rank three-way RNN dynamics problem.

Reference recurrence (per batch element, HID=512, RANK=4, T=1000):
    r_t = tanh(x_t + bias)
    x_{t+1} = x_t + 0.05*n_t + TAU*(-x_t + hidden_t + u_t @ W_in.T)
    hidden_t = ((r_t @ M) * (r_t @ N)) @ L.T / HID^2
outputs:
    trajectories = [x_0 .. x_1000]           (B, T+1, HID)
    output_t = tanh(x_{t+1}) @ W_out.T       (B, T, OUT)
    x_final = x_1000                         (B, HID)

Strategy: data-parallel over batch across 8 cores (16 batch rows/core).
Per-core device layout keeps HID on partitions: state tiles are
[128 partitions, 4 hid-chunks x 16 batch] fp32.

Per step (serial chain, Tile framework handles sync):
  PE : psum1[11,16] = sum_c [M+N | M-N | W_out^T]_c^T @ r[:, c]   (4 acc matmuls)
  ACT: z[8,16]  = Square(psum1[0:8])          # rM*rN = ((s^2 - d^2))/4, 1/4 folded in L
  PE : xq[128,64] = [L' | -L']_c^T @ z        (4 matmuls, K=8)
  DVE: lin = 0.8*x_prev + npre_t              (fused scalar_tensor_tensor, off-chain)
  DVE: x   = xq + lin                         -> SBUF staging block
  ACT: r   = tanh(x)
  DVE: out_sb[:, t] = psum1[8:11]             (output projection rides psum1 for free)

Host folds u @ W_in.T, the 0.05 noise scale, and the bias shift into a
single pre-computed per-step additive term `npre`; x is tracked as
y = x + bias (exact refold, bias subtracted on host afterward).
"""

import os

import numpy as np

HID = 512
RANK = 4
IN_SIZE = 3
OUT_SIZE = 3
BATCH = 128
SEQ = 1000
NOISE_STD = 0.05
TAU = 0.2

NCORES = 8
BLOC = BATCH // NCORES  # 16 batch rows per core
CH = HID // 128  # 4 hid chunks
COL = CH * BLOC  # 64 free columns per state tile
NB = 50  # timesteps per DMA staging block

_CACHE = {}


def _build_nc(T, nb=NB):
    import concourse.bacc as bacc
    import concourse.mybir as mybir
    from concourse import tile

    fp32 = mybir.dt.float32
    Act = mybir.ActivationFunctionType
    Alu = mybir.AluOpType

    nc = bacc.Bacc(
        "TRN2",
        target_bir_lowering=False,
        debug=False,
        enable_asserts=False,
        num_devices=NCORES,
    )

    npre_d = nc.dram_tensor("npre", [128, T * COL], fp32, kind="ExternalInput")
    y0_d = nc.dram_tensor("y0", [128, COL], fp32, kind="ExternalInput")
    r0_d = nc.dram_tensor("r0", [128, COL], fp32, kind="ExternalInput")
    wmno_d = nc.dram_tensor("wmno", [128, CH * 11], fp32, kind="ExternalInput")
    wproj_d = nc.dram_tensor("wproj", [8, HID], fp32, kind="ExternalInput")
    traj_d = nc.dram_tensor("traj", [128, T * COL], fp32, kind="ExternalOutput")
    out_d = nc.dram_tensor("outp", [OUT_SIZE, T * BLOC], fp32, kind="ExternalOutput")

    nblocks = (T + nb - 1) // nb
    assert T % nb == 0

    with tile.TileContext(nc) as tc:
        with (
            tc.tile_pool(name="consts", bufs=1) as cpool,
            tc.tile_pool(name="npre", bufs=2) as npool,
            tc.tile_pool(name="xstage", bufs=2) as xpool,
            tc.tile_pool(name="rtile", bufs=2) as rpool,
            tc.tile_pool(name="ztile", bufs=2) as zpool,
            tc.tile_pool(name="lin", bufs=2) as lpool,
            tc.tile_pool(name="outsb", bufs=1) as opool,
            tc.tile_pool(name="psum1", bufs=2, space="PSUM") as p1pool,
            tc.tile_pool(name="psum2", bufs=2, space="PSUM") as p2pool,
        ):
            wmno = cpool.tile([128, CH * 11], fp32)
            nc.sync.dma_start(wmno[:], wmno_d[:])
            wproj = cpool.tile([8, HID], fp32)
            nc.sync.dma_start(wproj[:], wproj_d[:])
            y0 = cpool.tile([128, COL], fp32)
            nc.sync.dma_start(y0[:], y0_d[:])
            r0 = cpool.tile([128, COL], fp32)
            nc.sync.dma_start(r0[:], r0_d[:])

            out_sb = opool.tile([OUT_SIZE, T * BLOC], fp32)

            # npre block prefetch (split each block DMA 4 ways across queues)
            def fetch_npre(blk):
                t0 = blk * nb * COL
                tl = npool.tile([128, nb * COL], fp32, tag="npre")
                q = nb * COL // 4
                for i in range(4):
                    nc.sync.dma_start(
                        tl[:, i * q : (i + 1) * q],
                        npre_d[:, t0 + i * q : t0 + (i + 1) * q],
                    )
                return tl

            npre_cur = fetch_npre(0)
            npre_next = None

            # initial psum1 from r0
            p1 = p1pool.tile([11, BLOC], fp32)
            for c in range(CH):
                nc.tensor.matmul(
                    p1[:],
                    wmno[:, c * 11 : (c + 1) * 11],
                    r0[:, c * BLOC : (c + 1) * BLOC],
                    start=(c == 0),
                    stop=(c == CH - 1),
                )

            x_prev = y0[:]
            xstage = None
            for t in range(T):
                blk, off = divmod(t, nb)
                if off == 0:
                    if blk > 0:
                        npre_cur = npre_next
                    if blk + 1 < nblocks:
                        npre_next = fetch_npre(blk + 1)
                    xstage = xpool.tile([128, nb * COL], fp32, tag="xstage")

                # z = Square(s|d)  [8,16]
                z = zpool.tile([8, BLOC], fp32)
                nc.scalar.activation(z[:], p1[0:8, :], Act.Square)

                # xq = projection back to hid space, one matmul per chunk
                xq = p2pool.tile([128, COL], fp32)
                for c in range(CH):
                    nc.tensor.matmul(
                        xq[:, c * BLOC : (c + 1) * BLOC],
                        wproj[:, c * 128 : (c + 1) * 128],
                        z[:],
                        start=True,
                        stop=True,
                    )

                # lin = 0.8 * x_prev + npre_t   (off critical chain)
                lin = lpool.tile([128, COL], fp32)
                nc.vector.scalar_tensor_tensor(
                    lin[:],
                    x_prev,
                    1.0 - TAU,
                    npre_cur[:, off * COL : (off + 1) * COL],
                    Alu.mult,
                    Alu.add,
                )

                # x = xq + lin -> SBUF staging
                xcur = xstage[:, off * COL : (off + 1) * COL]
                nc.vector.scalar_tensor_tensor(
                    xcur, xq[:], 1.0, lin[:], Alu.mult, Alu.add
                )

                # r = tanh(x)
                r = rpool.tile([128, COL], fp32)
                nc.scalar.activation(r[:], xcur, Act.Tanh)

                # psum1 for step t+1 (also carries output row for step t)
                p1 = p1pool.tile([11, BLOC], fp32)
                for c in range(CH):
                    nc.tensor.matmul(
                        p1[:],
                        wmno[:, c * 11 : (c + 1) * 11],
                        r[:, c * BLOC : (c + 1) * BLOC],
                        start=(c == 0),
                        stop=(c == CH - 1),
                    )

                # output row for time t  (W_out @ r_{t+1})
                nc.vector.tensor_copy(
                    out_sb[:, t * BLOC : (t + 1) * BLOC], p1[8:11, :]
                )

                x_prev = xcur
                if off == nb - 1:
                    t0 = blk * nb * COL
                    q = nb * COL // 4
                    for i in range(4):
                        nc.sync.dma_start(
                            traj_d[:, t0 + i * q : t0 + (i + 1) * q],
                            xstage[:, i * q : (i + 1) * q],
                        )

            nc.sync.dma_start(out_d[:], out_sb[:])

    nc.compile()
    return nc


def _get_nc(T):
    if T not in _CACHE:
        _CACHE[T] = _build_nc(T)
    return _CACHE[T]


def _to_dev_layout(a):
    """(B_LOC, HID) -> [128, COL] with col = chunk*BLOC + b."""
    return np.ascontiguousarray(
        a.reshape(BLOC, CH, 128).transpose(2, 1, 0).reshape(128, COL)
    )


def prepare_in_maps(u, x0, noise, L, M, N, bias_tensor, W_in, W_out, T):
    u = np.asarray(u, np.float32)
    x0 = np.asarray(x0, np.float32)
    noise = np.asarray(noise, np.float32)
    L = np.asarray(L, np.float32)
    M = np.asarray(M, np.float32)
    N = np.asarray(N, np.float32)
    bias = np.asarray(bias_tensor, np.float32)
    W_in = np.asarray(W_in, np.float32)
    W_out = np.asarray(W_out, np.float32)

    # weights: psum1 lhsT per chunk: [M+N | M-N | W_out^T]
    wmno = np.empty((128, CH * 11), np.float32)
    s = M + N
    d = M - N
    for c in range(CH):
        rows = slice(c * 128, (c + 1) * 128)
        wmno[:, c * 11 : c * 11 + 4] = s[rows]
        wmno[:, c * 11 + 4 : c * 11 + 8] = d[rows]
        wmno[:, c * 11 + 8 : c * 11 + 11] = W_out[:, rows].T
    # projection lhsT: rows 0:4 = L2^T, 4:8 = -L2^T with scales folded
    L2 = (TAU * 0.25 / (HID * HID)) * L
    wproj = np.concatenate([L2.T, -L2.T], axis=0).astype(np.float32)
    wproj = np.ascontiguousarray(wproj)

    in_maps = []
    for k in range(NCORES):
        bs = slice(k * BLOC, (k + 1) * BLOC)
        u_k = u[bs]  # (BLOC, T, IN)
        noise_k = noise[:, bs, :]  # (T, BLOC, HID)
        # npre[t,b,h] = 0.05*noise + TAU*(u@W_in^T) + TAU*bias
        npre = NOISE_STD * noise_k
        npre += TAU * np.einsum("bti,hi->tbh", u_k, W_in, optimize=True)
        npre += TAU * bias
        npre_dev = np.ascontiguousarray(
            npre.reshape(T, BLOC, CH, 128).transpose(3, 0, 2, 1)
        ).reshape(128, T * COL)
        y0 = _to_dev_layout(x0[bs] + bias)
        r0 = _to_dev_layout(np.tanh(x0[bs]))
        in_maps.append(
            {
                "npre": npre_dev,
                "y0": y0,
                "r0": r0,
                "wmno": wmno,
                "wproj": wproj,
            }
        )
    return in_maps


def assemble_outputs(results, x0, bias_tensor, T):
    """results: list per core of {traj:[128,T*COL], outp:[3,T*BLOC]}."""
    x0 = np.asarray(x0, np.float32)
    bias = np.asarray(bias_tensor, np.float32)
    output = np.empty((BATCH, T, OUT_SIZE), np.float32)
    traj = np.empty((BATCH, T + 1, HID), np.float32)
    traj[:, 0, :] = x0
    for k, res in enumerate(results):
        bs = slice(k * BLOC, (k + 1) * BLOC)
        td = res["traj"].reshape(128, T, CH, BLOC)
        traj[bs, 1:, :] = td.transpose(3, 1, 2, 0).reshape(BLOC, T, HID) - bias
        od = res["outp"].reshape(OUT_SIZE, T, BLOC)
        output[bs] = od.transpose(2, 1, 0)
    x_final = np.ascontiguousarray(traj[:, T, :])
    return output, x_final, traj


def run_on_hw(in_maps, T, trace=False):
    from concourse.bass_utils import run_bass_kernel_spmd

    nc = _get_nc(T)
    res = run_bass_kernel_spmd(
        nc, in_maps, core_ids=list(range(NCORES)), trace=trace
    )
    return res


def kernel(u, x0, noise, L, M, N, bias_tensor, W_in, W_out):
    T = u.shape[1]
    in_maps = prepare_in_maps(u, x0, noise, L, M, N, bias_tensor, W_in, W_out, T)
    res = run_on_hw(in_maps, T, trace=False)
    return assemble_outputs(res.results, x0, bias_tensor, T)


# revision 4
# speedup vs baseline: 28.3409x; 28.3409x over previous
"""Trainium2 Bass kernel for the low-rank three-way RNN dynamics problem.

Reference recurrence (per batch element, HID=512, RANK=4, T=1000):
    r_t = tanh(x_t + bias)
    x_{t+1} = x_t + 0.05*n_t + TAU*(-x_t + hidden_t + u_t @ W_in.T)
    hidden_t = ((r_t @ M) * (r_t @ N)) @ L.T / HID^2
outputs:
    trajectories = [x_0 .. x_1000]           (B, T+1, HID)
    output_t = tanh(x_{t+1}) @ W_out.T       (B, T, OUT)
    x_final = x_1000                         (B, HID)

Strategy: data-parallel over batch across 8 cores (16 batch rows/core).
Per-core device layout keeps HID on partitions: state tiles are
[128 partitions, 4 hid-chunks x 16 batch] fp32.

Per step (serial chain, Tile framework handles sync):
  PE : psum1[11,16] = sum_c [M+N | M-N | W_out^T]_c^T @ r[:, c]   (4 acc matmuls)
  ACT: z[8,16]  = Square(psum1[0:8])          # rM*rN = ((s^2 - d^2))/4, 1/4 folded in L
  PE : xq[128,64] = [L' | -L']_c^T @ z        (4 matmuls, K=8)
  DVE: lin = 0.8*x_prev + npre_t              (fused scalar_tensor_tensor, off-chain)
  DVE: x   = xq + lin                         -> SBUF staging block
  ACT: r   = tanh(x)
  DVE: out_sb[:, t] = psum1[8:11]             (output projection rides psum1 for free)

Host folds u @ W_in.T, the 0.05 noise scale, and the bias shift into a
single pre-computed per-step additive term `npre`; x is tracked as
y = x + bias (exact refold, bias subtracted on host afterward).
"""

import os

import numpy as np

HID = 512
RANK = 4
IN_SIZE = 3
OUT_SIZE = 3
BATCH = 128
SEQ = 1000
NOISE_STD = 0.05
TAU = 0.2

NCORES = 8
BLOC = BATCH // NCORES  # 16 batch rows per core
CH = HID // 128  # 4 hid chunks
COL = CH * BLOC  # 64 free columns per state tile
NB = 50  # timesteps per DMA staging block

_CACHE = {}


def _build_nc(T, nb=NB, repeat=1):
    import concourse.bacc as bacc
    import concourse.mybir as mybir
    from concourse import tile

    fp32 = mybir.dt.float32
    Act = mybir.ActivationFunctionType
    Alu = mybir.AluOpType

    nc = bacc.Bacc(
        "TRN2",
        target_bir_lowering=False,
        debug=False,
        enable_asserts=False,
        num_devices=NCORES,
    )

    npre_d = nc.dram_tensor("npre", [128, T * COL], fp32, kind="ExternalInput")
    y0_d = nc.dram_tensor("y0", [128, COL], fp32, kind="ExternalInput")
    r0_d = nc.dram_tensor("r0", [128, COL], fp32, kind="ExternalInput")
    wmno_d = nc.dram_tensor("wmno", [128, CH * 35], fp32, kind="ExternalInput")
    wproj_d = nc.dram_tensor("wproj", [8, HID], fp32, kind="ExternalInput")
    traj_d = nc.dram_tensor("traj", [128, T * COL], fp32, kind="ExternalOutput")
    out_d = nc.dram_tensor("outp", [OUT_SIZE, T * BLOC], fp32, kind="ExternalOutput")

    nblocks = (T + nb - 1) // nb
    assert T % nb == 0

    with tile.TileContext(nc) as tc:
        with (
            tc.tile_pool(name="consts", bufs=1) as cpool,
            tc.tile_pool(name="npre", bufs=2) as npool,
            tc.tile_pool(name="xstage", bufs=2) as xpool,
            tc.tile_pool(name="rtile", bufs=2) as rpool,
            tc.tile_pool(name="ztile", bufs=2) as zpool,
            tc.tile_pool(name="lin", bufs=2) as lpool,
            tc.tile_pool(name="outsb", bufs=1) as opool,
            tc.tile_pool(name="psum1", bufs=2, space="PSUM") as p1pool,
            tc.tile_pool(name="psum2", bufs=3, space="PSUM") as p2pool,
        ):
            wmno = cpool.tile([128, CH * 35], fp32)
            nc.sync.dma_start(wmno[:], wmno_d[:])
            wproj = cpool.tile([8, HID], fp32)
            nc.sync.dma_start(wproj[:], wproj_d[:])
            y0 = cpool.tile([128, COL], fp32)
            nc.sync.dma_start(y0[:], y0_d[:])
            r0 = cpool.tile([128, COL], fp32)
            nc.sync.dma_start(r0[:], r0_d[:])

            out_sb = opool.tile([OUT_SIZE, T * BLOC], fp32)

            # zero constants for PSUM has_written bootstrap
            zw = cpool.tile([1, 128], fp32)
            nc.vector.memset(zw[:], 0.0)
            zr = cpool.tile([1, 512], fp32)
            nc.vector.memset(zr[:], 0.0)

            # npre block prefetch (split each block DMA 4 ways across queues)
            def fetch_npre(blk):
                t0 = blk * nb * COL
                tl = npool.tile([128, nb * COL], fp32, tag="npre")
                q = nb * COL // 4
                for i in range(4):
                    nc.sync.dma_start(
                        tl[:, i * q : (i + 1) * q],
                        npre_d[:, t0 + i * q : t0 + (i + 1) * q],
                    )
                return tl

            npre_cur = fetch_npre(0)
            npre_next = None

            # initial psum1 from r0
            p1 = p1pool.tile([35, BLOC], fp32)
            for c in range(CH):
                nc.tensor.matmul(
                    p1[:],
                    wmno[:, c * 35 : (c + 1) * 35],
                    r0[:, c * BLOC : (c + 1) * BLOC],
                    start=(c == 0),
                    stop=(c == CH - 1),
                )

            # bootstrap: one full-bank zero matmul per rotating xq slot so
            # has_written is set everywhere; later matmuls use start=False and
            # accumulate onto DVE-prewritten contents.
            boot = []
            for _ in range(3):
                bt = p2pool.tile([128, 512], fp32, tag="xq")
                nc.tensor.matmul(bt[:], zw[:], zr[:], start=True, stop=False,
                                 skip_group_check=True)
                boot.append(bt)

            x_prev = y0[:]
            xstage = None
            for tt in range(repeat * T):
                t = tt % T
                blk, off = divmod(t, nb)
                if off == 0:
                    if blk > 0:
                        npre_cur = npre_next
                    if blk + 1 < nblocks:
                        npre_next = fetch_npre(blk + 1)
                    xstage = xpool.tile([128, nb * COL], fp32, tag="xstage")

                # z = Square(s|d)  [8,16]
                z = zpool.tile([8, BLOC], fp32)
                nc.scalar.activation(z[:], p1[0:8, :], Act.Square)

                # xq = 0.8*x_prev + npre_t, pre-written by DVE into PSUM;
                # projection matmuls then accumulate on top (start=False).
                xq = p2pool.tile([128, COL], fp32, tag="xq")
                nc.vector.scalar_tensor_tensor(
                    xq[:],
                    x_prev,
                    1.0 - TAU,
                    npre_cur[:, off * COL : (off + 1) * COL],
                    Alu.mult,
                    Alu.add,
                )
                for c in range(CH):
                    nc.tensor.matmul(
                        xq[:, c * BLOC : (c + 1) * BLOC],
                        wproj[:, c * 128 : (c + 1) * 128],
                        z[:],
                        start=False,
                        stop=False,
                        skip_group_check=True,
                    )

                # r = tanh(x)   (ACT reads PSUM directly)
                r = rpool.tile([128, COL], fp32)
                nc.scalar.activation(r[:], xq[:], Act.Tanh)

                # stage x for the trajectory DMA (off critical chain)
                xcur = xstage[:, off * COL : (off + 1) * COL]
                nc.vector.tensor_copy(xcur, xq[:])

                # psum1 for step t+1 (also carries output row for step t)
                p1 = p1pool.tile([35, BLOC], fp32)
                for c in range(CH):
                    nc.tensor.matmul(
                        p1[:],
                        wmno[:, c * 35 : (c + 1) * 35],
                        r[:, c * BLOC : (c + 1) * BLOC],
                        start=(c == 0),
                        stop=(c == CH - 1),
                    )

                # output row for time t  (W_out @ r_{t+1})
                nc.vector.tensor_copy(
                    out_sb[:, t * BLOC : (t + 1) * BLOC], p1[32:35, :]
                )

                x_prev = xq[:]
                if off == nb - 1:
                    t0 = blk * nb * COL
                    q = nb * COL // 4
                    for i in range(4):
                        nc.sync.dma_start(
                            traj_d[:, t0 + i * q : t0 + (i + 1) * q],
                            xstage[:, i * q : (i + 1) * q],
                        )

            nc.sync.dma_start(out_d[:], out_sb[:])

    nc.compile()
    return nc


def _get_nc(T):
    if T not in _CACHE:
        _CACHE[T] = _build_nc(T)
    return _CACHE[T]


def _to_dev_layout(a):
    """(B_LOC, HID) -> [128, COL] with col = chunk*BLOC + b."""
    return np.ascontiguousarray(
        a.reshape(BLOC, CH, 128).transpose(2, 1, 0).reshape(128, COL)
    )


def prepare_in_maps(u, x0, noise, L, M, N, bias_tensor, W_in, W_out, T):
    u = np.asarray(u, np.float32)
    x0 = np.asarray(x0, np.float32)
    noise = np.asarray(noise, np.float32)
    L = np.asarray(L, np.float32)
    M = np.asarray(M, np.float32)
    N = np.asarray(N, np.float32)
    bias = np.asarray(bias_tensor, np.float32)
    W_in = np.asarray(W_in, np.float32)
    W_out = np.asarray(W_out, np.float32)

    # weights: psum1 lhsT per chunk: [M+N | M-N | W_out^T]
    wmno = np.zeros((128, CH * 35), np.float32)
    s = M + N
    d = M - N
    for c in range(CH):
        rows = slice(c * 128, (c + 1) * 128)
        wmno[:, c * 35 : c * 35 + 4] = s[rows]
        wmno[:, c * 35 + 4 : c * 35 + 8] = d[rows]
        wmno[:, c * 35 + 32 : c * 35 + 35] = W_out[:, rows].T
    # projection lhsT: rows 0:4 = L2^T, 4:8 = -L2^T with scales folded
    L2 = (TAU * 0.25 / (HID * HID)) * L
    wproj = np.concatenate([L2.T, -L2.T], axis=0).astype(np.float32)
    wproj = np.ascontiguousarray(wproj)

    in_maps = []
    for k in range(NCORES):
        bs = slice(k * BLOC, (k + 1) * BLOC)
        u_k = u[bs]  # (BLOC, T, IN)
        noise_k = noise[:, bs, :]  # (T, BLOC, HID)
        # npre[t,b,h] = 0.05*noise + TAU*(u@W_in^T) + TAU*bias
        npre = NOISE_STD * noise_k
        npre += TAU * np.einsum("bti,hi->tbh", u_k, W_in, optimize=True)
        npre += TAU * bias
        npre_dev = np.ascontiguousarray(
            npre.reshape(T, BLOC, CH, 128).transpose(3, 0, 2, 1)
        ).reshape(128, T * COL)
        y0 = _to_dev_layout(x0[bs] + bias)
        r0 = _to_dev_layout(np.tanh(x0[bs]))
        in_maps.append(
            {
                "npre": npre_dev,
                "y0": y0,
                "r0": r0,
                "wmno": wmno,
                "wproj": wproj,
            }
        )
    return in_maps


def assemble_outputs(results, x0, bias_tensor, T):
    """results: list per core of {traj:[128,T*COL], outp:[3,T*BLOC]}."""
    x0 = np.asarray(x0, np.float32)
    bias = np.asarray(bias_tensor, np.float32)
    output = np.empty((BATCH, T, OUT_SIZE), np.float32)
    traj = np.empty((BATCH, T + 1, HID), np.float32)
    traj[:, 0, :] = x0
    for k, res in enumerate(results):
        bs = slice(k * BLOC, (k + 1) * BLOC)
        td = res["traj"].reshape(128, T, CH, BLOC)
        traj[bs, 1:, :] = td.transpose(3, 1, 2, 0).reshape(BLOC, T, HID) - bias
        od = res["outp"].reshape(OUT_SIZE, T, BLOC)
        output[bs] = od.transpose(2, 1, 0)
    x_final = np.ascontiguousarray(traj[:, T, :])
    return output, x_final, traj


def run_on_hw(in_maps, T, trace=False):
    from concourse.bass_utils import run_bass_kernel_spmd

    nc = _get_nc(T)
    res = run_bass_kernel_spmd(
        nc, in_maps, core_ids=list(range(NCORES)), trace=trace
    )
    return res


def kernel(u, x0, noise, L, M, N, bias_tensor, W_in, W_out):
    T = u.shape[1]
    in_maps = prepare_in_maps(u, x0, noise, L, M, N, bias_tensor, W_in, W_out, T)
    res = run_on_hw(in_maps, T, trace=False)
    return assemble_outputs(res.results, x0, bias_tensor, T)
